# revision 1
# baseline (speedup 1.0000x reference)
"""Sliding-window causal self-attention for Trainium2, 8 NeuronCores.

Problem: B=4, T=2048, C=1024, 16 heads x 64 dim, window=256 causal band.
  qkv = x @ W_qkv.T ; windowed-causal attention ; out = y @ W_proj.T

Sharding: 8 cores = 4 batches x 2 sequence halves (1024 queries each).
Each core receives a 1280-row "extended" slice of its batch's x (256
preceding rows for the attention window, zero-padded for the first half)
and computes its 1024 output rows with zero cross-core communication.

On-core layout is fully transposed (channel-major):
  - host passes x_ext^T [C, 1280], W_qkv^T [C, 3C], W_proj^T [C, C]
  - q^T,k^T computed head-dim-major: [c, t];  V in natural [t, c] layout,
    augmented with a ones column per head (65 cols/head) so the AV matmul
    also produces the softmax denominator (row 64 of each PSUM tile).
  - scores computed as S^T[key, query] per 256-query chunk over its
    512-key window (4 key subtiles of 128); exp on ScalarE (scale=1/8
    folded in); band masks are 0/1 input tensors applied multiplicatively
    post-exp on VectorE; AV accumulates over the 4 subtiles.
  - y^T/rowsum -> reciprocal -> normalize -> proj -> outT [C, 1024];
    host transposes back.

All matmuls use float32r (TF32-like, ~1e-4 rel err, full PE speed at
moving-dim >= 256).
"""

import numpy as np
from contextlib import ExitStack

import concourse.bass as bass
import concourse.tile as tile
import concourse.mybir as mybir
from concourse import bacc
from concourse.tile import add_dep_helper
from concourse import bass_utils

F32 = mybir.dt.float32
F32R = mybir.dt.float32r
AF = mybir.ActivationFunctionType

C = 1024
HEADS = 16
D = 64
WINDOW = 256
QC = 256            # queries per chunk
PAD = WINDOW        # leading ext rows
CO = C // 128       # 8

FULL_MASK = False   # debug: disable half-width exp/mask scheme
PAIR_S = False       # debug: share one PSUM bank between two heads' scores


def _build_body(tc, xT, wqkvT, wprojT, masks, outT, TQ):
    nc = tc.nc
    EXT = TQ + PAD
    EO = EXT // 128
    NCH = TQ // QC

    with ExitStack() as outer:
        kt_pool = outer.enter_context(tc.tile_pool(name="kt", bufs=1))
        qt_pool = outer.enter_context(tc.tile_pool(name="qt", bufs=1))
        v_pool = outer.enter_context(tc.tile_pool(name="vv", bufs=1))
        const_pool = outer.enter_context(tc.tile_pool(name="const", bufs=1))

        kT = kt_pool.tile([128, CO, EXT], F32R)
        qT = qt_pool.tile([128, CO, TQ], F32R)
        V = v_pool.tile([128, EO, HEADS * (D + 1)], F32R)
        ones_col = const_pool.tile([128, 1], F32)
        nc.vector.memset(ones_col[:], 1.0)
        zf = const_pool.tile([128, 1], F32)
        nc.vector.memset(zf[:], 0.0)

        # ones columns of V_aug (col D of each head's 65-col group)
        v_ones_view = V[:].rearrange("p e (h x) -> p e h x", x=D + 1)[:, :, :, D]
        nc.vector.tensor_copy(
            v_ones_view, ones_col[:, 0:1].broadcast_to([128, EO, HEADS])
        )

        # ---------------- Phase A: qkv projections ----------------
        with ExitStack() as ctx:
            x_pool = ctx.enter_context(tc.tile_pool(name="xx", bufs=1))
            w_pool = ctx.enter_context(tc.tile_pool(name="wA", bufs=2))
            psA = ctx.enter_context(tc.tile_pool(name="psA", bufs=4, space="PSUM"))

            xt = x_pool.tile([128, CO, EXT], F32R)
            xTr = xT.rearrange("(o p) t -> p o t", p=128)

            # front-loaded x DMA, finest slices first so co=0 starts early
            nc.sync.dma_start(xt[:, :, 0:128], xTr[:, :, 0:128])

            def dma_w(dst, col0):
                nc.sync.dma_start(
                    dst, wqkvT[:, col0 : col0 + 128].rearrange("(o p) c -> p o c", p=128)
                )

            wk = {}
            wq = {}
            wk[0] = w_pool.tile([128, CO, 128], F32R, tag="wk", name="wk0")
            dma_w(wk[0], C)  # k block, co=0
            nc.sync.dma_start(xt[:, :, 128:512], xTr[:, :, 128:512])
            nc.sync.dma_start(xt[:, :, 512:1024], xTr[:, :, 512:1024])
            wq[0] = w_pool.tile([128, CO, 128], F32R, tag="wq", name="wq0")
            dma_w(wq[0], 0)  # q block, co=0
            nc.sync.dma_start(xt[:, :, 1024:EXT], xTr[:, :, 1024:EXT])

            K_CHUNKS0 = [(0, 128), (128, 384), (512, 512), (1024, EXT - 1024)]
            K_CHUNKS = [(0, 512), (512, 512), (1024, EXT - 1024)]
            Q_CHUNKS = [(0, 512), (512, TQ - 512)]

            for co in range(CO):
                # prefetch next co's W blocks (double-buffered tags)
                if co + 1 < CO:
                    wk[co + 1] = w_pool.tile([128, CO, 128], F32R, tag="wk", name=f"wk{co+1}")
                    dma_w(wk[co + 1], C + (co + 1) * 128)
                    wq[co + 1] = w_pool.tile([128, CO, 128], F32R, tag="wq", name=f"wq{co+1}")
                    dma_w(wq[co + 1], (co + 1) * 128)
                for t0, tn in K_CHUNKS0 if co == 0 else K_CHUNKS:
                    ps = psA.tile([128, 512], F32, tag="ps")
                    for ki in range(CO):
                        nc.tensor.matmul(
                            ps[:, :tn],
                            wk[co][:, ki, :],
                            xt[:, ki, t0 : t0 + tn],
                            start=(ki == 0),
                            stop=(ki == CO - 1),
                        )
                    nc.scalar.activation(kT[:, co, t0 : t0 + tn], ps[:, :tn], AF.Copy)
                for t0, tn in Q_CHUNKS:
                    ps = psA.tile([128, 512], F32, tag="ps")
                    for ki in range(CO):
                        nc.tensor.matmul(
                            ps[:, :tn],
                            wq[co][:, ki, :],
                            xt[:, ki, PAD + t0 : PAD + t0 + tn],
                            start=(ki == 0),
                            stop=(ki == CO - 1),
                        )
                    nc.scalar.activation(qT[:, co, t0 : t0 + tn], ps[:, :tn], AF.Copy)

            # V natural layout [t, c], 256-col chunks (4 heads each)
            for cb in range(4):
                wv = w_pool.tile([128, CO, 256], F32R, tag="wv")
                nc.sync.dma_start(
                    wv[:],
                    wqkvT[:, 2 * C + cb * 256 : 2 * C + (cb + 1) * 256].rearrange(
                        "(o p) c -> p o c", p=128
                    ),
                )
                for eo in range(EO):
                    ps = psA.tile([128, 256], F32, tag="psv")
                    for ki in range(CO):
                        nc.tensor.matmul(
                            ps[:],
                            xt[:, ki, eo * 128 : (eo + 1) * 128],
                            wv[:, ki, :],
                            start=(ki == 0),
                            stop=(ki == CO - 1),
                        )
                    v_dst = V[:].rearrange("p e (h x) -> p e h x", x=D + 1)[
                        :, eo, 4 * cb : 4 * cb + 4, 0:D
                    ]
                    nc.scalar.activation(
                        v_dst, ps[:].rearrange("p (h d) -> p h d", d=D), AF.Copy
                    )

        # ---------------- Phase B: attention + projection ----------------
        with ExitStack() as ctx:
            mask_sb = const_pool.tile([128, 8, QC], F32)
            zeros_qc = const_pool.tile([128, QC], F32R)
            nc.vector.tensor_copy(zeros_qc[:], zf[:, 0:1].broadcast_to([128, QC]))
            wp_pool = ctx.enter_context(tc.tile_pool(name="wp", bufs=1))
            pm_pool = ctx.enter_context(tc.tile_pool(name="pm", bufs=10))
            ostage_pool = ctx.enter_context(tc.tile_pool(name="ost", bufs=2))
            yu_pool = ctx.enter_context(tc.tile_pool(name="yu", bufs=4))
            ysb_pool = ctx.enter_context(tc.tile_pool(name="ysb", bufs=2))
            r_pool = ctx.enter_context(tc.tile_pool(name="rr", bufs=4))
            psS = ctx.enter_context(tc.tile_pool(name="psS", bufs=2, space="PSUM"))
            psY = ctx.enter_context(tc.tile_pool(name="psY", bufs=3, space="PSUM"))
            psP = ctx.enter_context(tc.tile_pool(name="psP", bufs=1, space="PSUM"))

            nc.sync.dma_start(mask_sb[:], masks.rearrange("m s p q -> p (m s) q"))
            wp_sb = wp_pool.tile([128, CO, C], F32R)
            nc.sync.dma_start(wp_sb[:], wprojT.rearrange("(o p) c -> p o c", p=128))

            for ch in range(NCH):
                mset = 0 if ch == 0 else 4  # mask set index base
                y_sb = ysb_pool.tile([128, CO, QC], F32R)

                for hp in range(8):
                    h0, h1 = 2 * hp, 2 * hp + 1
                    pms = {}
                    for s in range(4):
                        # two heads share one PSUM bank: head h1's matmul uses
                        # start=False (+skip_group_check) so it overwrites its
                        # untouched half without re-arming the bank's
                        # pending-zero region.
                        if PAIR_S:
                            Sp = psS.tile([128, 2, QC], F32, tag="S")
                            s_views = [Sp[:, 0, :], Sp[:, 1, :]]
                        else:
                            Sa = psS.tile([128, QC], F32, tag="Sa")
                            Sb = psS.tile([128, QC], F32, tag="Sb")
                            Sp = None
                            s_views = [Sa[:], Sb[:]]
                        mm_prev = None
                        for j, h in ((0, h0), (1, h1)):
                            pb = 64 * (h % 2)
                            coh = h // 2
                            mm = nc.tensor.matmul(
                                s_views[j],
                                kT[pb : pb + 64, coh, ch * QC + s * 128 : ch * QC + (s + 1) * 128],
                                qT[pb : pb + 64, coh, ch * QC : (ch + 1) * QC],
                                start=(j == 0 or not PAIR_S),
                                stop=True,
                                skip_group_check=(j == 1 and PAIR_S),
                            )
                            if PAIR_S and j == 1:
                                # the start=True matmul must clear the bank's
                                # has_written bits BEFORE the start=False one
                                # lands; they touch disjoint halves so no data
                                # dep exists -- order them explicitly.
                                add_dep_helper(
                                    mm.ins, mm_prev.ins, sync=True,
                                    reason="paired-S bank: start-clear first",
                                )
                            mm_prev = mm
                        PM = pm_pool.tile([128, 2, QC], F32R, tag="PM")
                        H = QC // 2
                        if ch == 0 or FULL_MASK:
                            # chunk 0: data-dependent masks (sequence start on
                            # first-half cores) -> full-width exp + mask
                            if PAIR_S:
                                nc.scalar.activation(PM[:], Sp[:], AF.Exp, scale=0.125)
                            else:
                                for j in (0, 1):
                                    nc.scalar.activation(
                                        PM[:, j, :], s_views[j], AF.Exp, scale=0.125
                                    )
                            nc.vector.tensor_tensor(
                                out=PM[:],
                                in0=PM[:],
                                in1=mask_sb[:, mset + s, :].unsqueeze(1).broadcast_to(
                                    [128, 2, QC]
                                ),
                                op=mybir.AluOpType.mult,
                            )
                        elif s in (0, 3):
                            # band mask kills one column half outright; exp
                            # only the live half, zero-fill the dead half
                            lo = 0 if s == 0 else H
                            dead = H if s == 0 else 0
                            for j in (0, 1):
                                nc.scalar.activation(
                                    PM[:, j, lo : lo + H],
                                    s_views[j][:, lo : lo + H],
                                    AF.Exp,
                                    scale=0.125,
                                )
                            nc.vector.tensor_tensor(
                                out=PM[:, :, lo : lo + H],
                                in0=PM[:, :, lo : lo + H],
                                in1=mask_sb[:, mset + s, lo : lo + H]
                                .unsqueeze(1)
                                .broadcast_to([128, 2, H]),
                                op=mybir.AluOpType.mult,
                            )
                            nc.vector.tensor_copy(
                                PM[:, :, dead : dead + H],
                                zeros_qc[:].rearrange("p (a b) -> p a b", a=2),
                            )
                        else:
                            # s=1: only cols [H:) touch the band edge;
                            # s=2: only cols [0:H). The other half is fully
                            # valid -> exp straight into PM, no mask there.
                            lo = H if s == 1 else 0
                            for j in (0, 1):
                                nc.scalar.activation(
                                    PM[:, j, :], s_views[j], AF.Exp, scale=0.125
                                )
                            nc.vector.tensor_tensor(
                                out=PM[:, :, lo : lo + H],
                                in0=PM[:, :, lo : lo + H],
                                in1=mask_sb[:, mset + s, lo : lo + H]
                                .unsqueeze(1)
                                .broadcast_to([128, 2, H]),
                                op=mybir.AluOpType.mult,
                            )
                        pms[s] = PM

                    for j, h in ((0, h0), (1, h1)):
                        yps = psY.tile([65, QC], F32, tag="y")
                        for s in range(4):
                            nc.tensor.matmul(
                                yps[:],
                                V[:, 2 * ch + s, (D + 1) * h : (D + 1) * (h + 1)],
                                pms[s][:, j, :],
                                start=(s == 0),
                                stop=(s == 3),
                            )
                        # rowsum row 64 -> SBUF (custom DVE ops can't read
                        # PSUM) and unnormalized y -> SBUF staging, freeing
                        # the PSUM tile immediately; then recip, broadcast,
                        # normalize out of SBUF.
                        pb = 64 * (h % 2)
                        rt = r_pool.tile([1, QC], F32, tag="rt")
                        nc.scalar.activation(rt[:], yps[64:65, :], AF.Copy)
                        yu = yu_pool.tile([128, QC], F32, tag="yu")
                        nc.vector.tensor_copy(yu[pb : pb + 64, :], yps[0:64, :])
                        Rr = r_pool.tile([1, QC], F32, tag="Rr")
                        nc.vector.reciprocal_approx_fast(Rr[:], rt[0:1, :])
                        rb = r_pool.tile([128, QC], F32, tag="rb")
                        nc.gpsimd.partition_broadcast(rb[:], Rr[0:1, :], channels=128)
                        nc.vector.tensor_tensor(
                            out=y_sb[pb : pb + 64, h // 2, :],
                            in0=yu[pb : pb + 64, :],
                            in1=rb[pb : pb + 64, :],
                            op=mybir.AluOpType.mult,
                        )

                for co in range(CO):
                    pps = psP.tile([128, QC], F32, tag="pj")
                    for ci in range(CO):
                        nc.tensor.matmul(
                            pps[:],
                            wp_sb[:, ci, co * 128 : (co + 1) * 128],
                            y_sb[:, ci, :],
                            start=(ci == 0),
                            stop=(ci == CO - 1),
                        )
                    o_sb = ostage_pool.tile([128, QC], F32, tag="ost")
                    nc.scalar.activation(o_sb[:], pps[:], AF.Copy)
                    nc.sync.dma_start(
                        outT[co * 128 : (co + 1) * 128, ch * QC : (ch + 1) * QC],
                        o_sb[:],
                    )


_BUILD_CACHE = {}


def build_bass(TQ=1024, enable_asserts=False):
    key = (TQ, enable_asserts)
    if key in _BUILD_CACHE:
        return _BUILD_CACHE[key]
    EXT = TQ + PAD
    nc = bacc.Bacc(
        "TRN2",
        target_bir_lowering=False,
        debug=False,
        enable_asserts=enable_asserts,
    )
    xT = nc.dram_tensor("xT", [C, EXT], F32R, kind="ExternalInput").ap()
    wqkvT = nc.dram_tensor("wqkvT", [C, 3 * C], F32R, kind="ExternalInput").ap()
    wprojT = nc.dram_tensor("wprojT", [C, C], F32R, kind="ExternalInput").ap()
    masks = nc.dram_tensor("masks", [2, 4, 128, QC], F32, kind="ExternalInput").ap()
    outT = nc.dram_tensor("outT", [C, TQ], F32, kind="ExternalOutput").ap()

    with tile.TileContext(nc) as tc:
        _build_body(tc, xT, wqkvT, wprojT, masks, outT, TQ)
    nc.compile()
    _BUILD_CACHE[key] = nc
    return nc


def make_masks(first_half: bool) -> np.ndarray:
    """0/1 band masks, [2 sets, 4 subtiles, 128 kr, QC qq].

    valid(kr, qq, s):  qq+1 <= 128*s+kr <= qq+256.
    Set 0 is used by chunk 0 (subtiles 0,1 zeroed on first-half cores:
    those keys fall before the sequence start); set 1 by chunks 1+.
    """
    kr = np.arange(128)[:, None]
    qq = np.arange(QC)[None, :]
    m = np.zeros((2, 4, 128, QC), dtype=np.float32)
    for s in range(4):
        kl = 128 * s + kr
        m[1, s] = ((qq + 1 <= kl) & (kl <= qq + 256)).astype(np.float32)
    m[0] = m[1]
    if first_half:
        m[0, 0] = 0.0
        m[0, 1] = 0.0
    return m


def _prep_core_inputs(x, wqkvT, wprojT, masks_by_half, b, half, TQ):
    start = half * TQ
    T = x.shape[1]
    ext = np.zeros((TQ + PAD, C), dtype=np.float32)
    lo = start - PAD
    src_lo = max(lo, 0)
    ext[src_lo - lo : TQ + PAD] = x[b, src_lo : start + TQ]
    return {
        "xT": np.ascontiguousarray(ext.T),
        "wqkvT": wqkvT,
        "wprojT": wprojT,
        "masks": masks_by_half[half],
    }


def kernel(x, W_qkv, W_proj):
    x = np.asarray(x, dtype=np.float32)
    W_qkv = np.asarray(W_qkv, dtype=np.float32)
    W_proj = np.asarray(W_proj, dtype=np.float32)
    B, T, _ = x.shape
    TQ = T // 2

    nc = build_bass(TQ)
    wqkvT = np.ascontiguousarray(W_qkv.T)
    wprojT = np.ascontiguousarray(W_proj.T)
    masks_by_half = {0: make_masks(True), 1: make_masks(False)}

    in_maps = [
        _prep_core_inputs(x, wqkvT, wprojT, masks_by_half, core // 2, core % 2, TQ)
        for core in range(8)
    ]
    res = bass_utils.run_bass_kernel_spmd(nc, in_maps, core_ids=list(range(8)))
    kernel.last_run_results = res

    out = np.empty((B, T, C), dtype=np.float32)
    for core in range(8):
        b, half = core // 2, core % 2
        out[b, half * TQ : (half + 1) * TQ, :] = res.results[core]["outT"].T
    return out



# revision 2
# speedup vs baseline: 1.0359x; 1.0359x over previous
"""Sliding-window causal self-attention for Trainium2, 8 NeuronCores. v2.

Problem: B=4, T=2048, C=1024, 16 heads x 64 dim, window=256 causal band.
  qkv = x @ W_qkv.T ; windowed-causal attention ; out = y @ W_proj.T

Sharding v2: 8 cores = 4 batches x 2 head-halves (8 heads each).
Each core computes q/k/v for its 8 heads over the FULL sequence (no
halo recompute), runs attention for all 2048 queries, and produces a
PARTIAL projection output (its heads' contribution, [1024, 2048] f32).
The host sums the two partials per batch and transposes.

All matmuls in bf16 (1 cycle/row at any moving width on TRN2's PE):
  - half-width (128-col) triangle matmuls: the s0 (upper) and s3
    (lower) key-subtiles of each 256-query chunk are valid for
    complementary query halves, so they share one PSUM tile at half
    width -> 25% fewer score/AV cycles.
  - band masks are two 128x128 triangles (mU/mL) applied
    multiplicatively post-exp on DVE in bf16 (2x DVE rate).
  - exp on ScalarE reads each score subtile straight from PSUM
    ([128,512] both heads at once), writes PM bf16.
  - AV uses V augmented with a ones column per head (65 cols) so row 64
    of the PSUM y tile is the softmax denominator; normalization is a
    DVE tensor_tensor reading y from PSUM, multiplying by the broadcast
    reciprocal, writing bf16 y_sb in one pass.

Scheduling: a flat work-item list interleaves phase-A GEMM groups of
column-period t with the attention chunks of period t-1 (full lag), so
phase B's scalar/vector chains hide under phase A's PE-heavy stretches;
within a chunk, AV(hp) is emitted one slot behind scores(hp+1) (lag-1)
to cover the exp->mask latency. Score subtiles flow sg2 -> sg1 -> sg03
and AV consumes them in that order so the first AV matmul only waits on
the earliest exp. Phase-A accumulation groups, score tiles, and proj
tiles all draw from shared one-bank PSUM rings (6 + 2 banks).
"""

import numpy as np
import ml_dtypes
from contextlib import ExitStack

import concourse.bass as bass
import concourse.tile as tile
import concourse.mybir as mybir
from concourse import bacc
from concourse.tile import add_dep_helper
from concourse import bass_utils

F32 = mybir.dt.float32
BF16 = mybir.dt.bfloat16
AF = mybir.ActivationFunctionType

B = 4
T = 2048
C = 1024
HL = 8              # heads per core
D = 64
QC = 256            # queries per attention chunk
KI = C // 128       # 8 contraction blocks for qkv
NCH = T // QC       # 8 attention chunks
TCN = 4             # phase-A column chunks of 512


def _build_body(tc, xT, wqkvT, wprojT, masks, outT):
    nc = tc.nc
    with ExitStack() as ctx:
        kq_pool = ctx.enter_context(tc.tile_pool(name="kq", bufs=1))
        w_pool = ctx.enter_context(tc.tile_pool(name="ww", bufs=1))
        x_pool = ctx.enter_context(tc.tile_pool(name="xx", bufs=1))
        const_pool = ctx.enter_context(tc.tile_pool(name="const", bufs=1))
        pm_pool = ctx.enter_context(tc.tile_pool(name="pm", bufs=3))
        r_pool = ctx.enter_context(tc.tile_pool(name="rr", bufs=3))
        ysb_pool = ctx.enter_context(tc.tile_pool(name="ysb", bufs=2))
        o_pool = ctx.enter_context(tc.tile_pool(name="ost", bufs=3))
        psB = ctx.enter_context(tc.tile_pool(name="psB", bufs=6, space="PSUM"))
        psY = ctx.enter_context(tc.tile_pool(name="psY", bufs=2, space="PSUM"))

        # kZ: per-head zero-padded K. Head h occupies partition rows
        # [64*(h%2), 64*(h%2)+64); the sibling half stays zero so a score
        # matmul can contract over all 128 partitions at tile position
        # (0,0) -- mixing PE tile positions within one PSUM bank faults
        # the hardware. The paired qT moving operand is used unmodified:
        # the zero stationary half annihilates the other head's term.
        kZ = kq_pool.tile([128, HL, T], BF16)
        qT = kq_pool.tile([128, 4, T], BF16)
        V = kq_pool.tile([128, 16, HL * (D + 1)], BF16)
        xt = x_pool.tile([128, KI, T], BF16)
        wq_sb = w_pool.tile([128, KI, 1536], BF16)
        wp_sb = w_pool.tile([128, 4, 1024], BF16)
        masks_sb = const_pool.tile([128, 2, 128], BF16)

        nc.gpsimd.memset(kZ[:], 0.0)
        ones_col = const_pool.tile([128, 1], F32)
        nc.vector.memset(ones_col[:], 1.0)
        v_ones_view = V[:].rearrange("p e (h x) -> p e h x", x=D + 1)[:, :, :, D]
        nc.vector.tensor_copy(
            v_ones_view, ones_col[:, 0:1].broadcast_to([128, 16, HL])
        )

        # ---- input DMAs: q/k weight cols + x chunk 0 first (finest useful
        # grain so the first GEMM group starts ASAP), v cols next ----
        xTr = xT.rearrange("(o p) t -> p o t", p=128)
        wqr = wqkvT.rearrange("(o p) c -> p o c", p=128)
        for ki in range(KI):
            nc.sync.dma_start(wq_sb[:, ki, 0:1024], wqr[:, ki, 0:1024])
            nc.sync.dma_start(xt[:, ki, 0:512], xTr[:, ki, 0:512])
        nc.sync.dma_start(masks_sb[:], masks.rearrange("m p q -> p m q"))
        for ki in range(KI):
            nc.sync.dma_start(wq_sb[:, ki, 1024:1536], wqr[:, ki, 1024:1536])
        for t in range(1, TCN):
            nc.sync.dma_start(xt[:, :, 512 * t : 512 * t + 512],
                              xTr[:, :, 512 * t : 512 * t + 512])
        nc.sync.dma_start(wp_sb[:], wprojT.rearrange("(o p) c -> p o c", p=128))

        # ================= work items =================
        def a_group(t, kind, i):
            # one qkv accumulation group: q/k block co=i, or v rows eo
            def run():
                c0 = 512 * t
                ps = psB.tile([128, 512], F32, tag="bank", name=f"A{kind}{t}_{i}")
                if kind in ("q", "k"):
                    wof = 128 * i if kind == "q" else 512 + 128 * i
                    for ki in range(KI):
                        nc.tensor.matmul(
                            ps[:],
                            wq_sb[:, ki, wof : wof + 128],
                            xt[:, ki, c0 : c0 + 512],
                            start=(ki == 0),
                            stop=(ki == KI - 1),
                        )
                    if kind == "q":
                        nc.vector.tensor_copy(qT[:, i, c0 : c0 + 512], ps[:])
                    else:
                        nc.scalar.activation(
                            kZ[0:64, 2 * i, c0 : c0 + 512], ps[0:64, :], AF.Copy)
                        nc.scalar.activation(
                            kZ[64:128, 2 * i + 1, c0 : c0 + 512], ps[64:128, :],
                            AF.Copy)
                else:  # v, eo = i
                    eo = i
                    for ki in range(KI):
                        nc.tensor.matmul(
                            ps[:],
                            xt[:, ki, 128 * eo : 128 * eo + 128],
                            wq_sb[:, ki, 1024:1536],
                            start=(ki == 0),
                            stop=(ki == KI - 1),
                        )
                    v_dst = V[:].rearrange("p e (h x) -> p e h x", x=D + 1)[
                        :, eo, :, 0:D
                    ]
                    nc.scalar.activation(
                        v_dst, ps[:].rearrange("p (h d) -> p h d", d=D), AF.Copy
                    )
            return run

        state = {}  # (ch, hp) -> PM tile; pch -> y_sb tile

        def sc_item(ch, hp):
            # scores + exp + masks for head pair hp of chunk ch
            def run():
                q0 = QC * ch
                PM = pm_pool.tile([128, 3, 2, QC], BF16, tag="PM",
                                  name=f"PM{ch}_{hp}")
                state[(ch, hp)] = PM
                PMf = PM[:].rearrange("p s j q -> p s (j q)")
                masks_flat = masks_sb[:].rearrange("p m q -> p (m q)")
                # sg order: 2 (oldest-ready), 1, 0(=s03 triangles)
                sgl = [2, 1, 0] if ch > 0 else [2, 0]
                for sg in sgl:
                    s = psB.tile([128, 2, QC], F32, tag="bank",
                                 name=f"S{sg}_{ch}_{hp}")
                    first = None
                    for j in (0, 1):
                        if sg == 0:
                            if True:
                                # ch==0: dummy scores vs keys block 0; they
                                # are never consumed (ch0's AV skips s03a and
                                # cols 0:128 of PM sg0 are unread) but they
                                # keep the PSUM bank fully written so the exp
                                # read never touches another tile's region.
                                klo = 256 * ch - 256 if ch > 0 else 0
                                mm = nc.tensor.matmul(
                                    s[:, j, 0:128],
                                    kZ[:, 2 * hp + j, klo : klo + 128],
                                    qT[:, hp, q0 : q0 + 128],
                                    start=first is None, stop=True,
                                    skip_group_check=first is not None,
                                )
                                if first is not None:
                                    add_dep_helper(mm.ins, first.ins, sync=True,
                                                   reason="bank clear order")
                                first = first or mm
                            mm = nc.tensor.matmul(
                                s[:, j, 128:256],
                                kZ[:, 2 * hp + j,
                                   256 * ch + 128 : 256 * ch + 256],
                                qT[:, hp, q0 + 128 : q0 + 256],
                                start=first is None, stop=True,
                                skip_group_check=first is not None,
                            )
                            if first is not None:
                                add_dep_helper(mm.ins, first.ins, sync=True,
                                               reason="bank clear order")
                            first = first or mm
                        else:
                            kb = 2 * ch - 2 + sg
                            mm = nc.tensor.matmul(
                                s[:, j, :],
                                kZ[:, 2 * hp + j, 128 * kb : 128 * kb + 128],
                                qT[:, hp, q0 : q0 + QC],
                                start=first is None, stop=True,
                                skip_group_check=first is not None,
                            )
                            if first is not None:
                                add_dep_helper(mm.ins, first.ins, sync=True,
                                               reason="bank clear order")
                            first = first or mm
                    nc.scalar.activation(
                        PMf[:, sg, :],
                        s[:].rearrange("p j q -> p (j q)"),
                        AF.Exp, scale=0.125,
                    )
                    # masks for this subtile group
                    if sg == 0:
                        nc.vector.tensor_tensor(
                            out=PM[:, 0, :, :],
                            in0=PM[:, 0, :, :],
                            in1=masks_flat.unsqueeze(1).broadcast_to(
                                [128, 2, QC]),
                            op=mybir.AluOpType.mult,
                        )
                    elif sg == 1:
                        nc.vector.tensor_tensor(
                            out=PM[:, 1, :, 128:256],
                            in0=PM[:, 1, :, 128:256],
                            in1=masks_sb[:, 0, :].unsqueeze(1).broadcast_to(
                                [128, 2, 128]),
                            op=mybir.AluOpType.mult,
                        )
                    else:
                        nc.vector.tensor_tensor(
                            out=PM[:, 2, :, 0:128],
                            in0=PM[:, 2, :, 0:128],
                            in1=masks_sb[:, 1, :].unsqueeze(1).broadcast_to(
                                [128, 2, 128]),
                            op=mybir.AluOpType.mult,
                        )
            return run

        def avn_item(ch, hp):
            # AV + rowsum/recip/broadcast/normalize for (ch, hp)
            def run():
                PM = state.pop((ch, hp))
                y_sb = state[("y", ch // 2)]
                yb = psY.tile([128, 512], F32, tag="yb", name=f"yb{ch}_{hp}")
                first = None
                for j in (0, 1):
                    hl = 2 * hp + j
                    qof = 256 * j

                    # the first matmul (start=True, unskipped) marks the bank's
                    # group open; the last (stop=True, unskipped) closes it so
                    # the rowsum/normalize reads pass the sim's group check.
                    # Everything in between uses skip_group_check.
                    def vmm(kb, pm_ap, qlo, stop, last=False):
                        nonlocal first
                        mm = nc.tensor.matmul(
                            yb[0:65, qof + qlo : qof + qlo + 128],
                            V[:, kb, 65 * hl : 65 * hl + 65],
                            pm_ap,
                            start=first is None,
                            # group bookkeeping: opener keeps the group open,
                            # only the unskipped `last` matmul closes it
                            stop=stop and first is not None,
                            skip_group_check=(first is not None) and not last,
                        )
                        if first is not None:
                            add_dep_helper(mm.ins, first.ins, sync=True,
                                           reason="bank clear order")
                        first = first or mm

                    # consume in exp-completion order: s2, s1, s03
                    if ch > 0:
                        vmm(2 * ch, PM[:, 2, j, 0:128], 0, False)
                        vmm(2 * ch, PM[:, 2, j, 128:256], 128, False)
                        vmm(2 * ch - 1, PM[:, 1, j, 0:128], 0, False)
                        vmm(2 * ch - 1, PM[:, 1, j, 128:256], 128, False)
                        vmm(2 * ch - 2, PM[:, 0, j, 0:128], 0, True)
                        vmm(2 * ch + 1, PM[:, 0, j, 128:256], 128, True,
                            last=(j == 1))
                    else:
                        vmm(2 * ch, PM[:, 2, j, 0:128], 0, True)
                        vmm(2 * ch, PM[:, 2, j, 128:256], 128, False)
                        vmm(2 * ch + 1, PM[:, 0, j, 128:256], 128, True,
                            last=(j == 1))

                rt = r_pool.tile([1, 512], F32, tag="rt", name=f"rt{ch}{hp}")
                nc.scalar.activation(rt[:], yb[64:65, :], AF.Copy)
                Rr = r_pool.tile([1, 512], F32, tag="Rr", name=f"Rr{ch}{hp}")
                nc.vector.reciprocal_approx_fast(Rr[:], rt[0:1, :])
                rb = r_pool.tile([128, 512], F32, tag="rb", name=f"rb{ch}{hp}")
                nc.gpsimd.partition_broadcast(rb[:], Rr[0:1, :], channels=128)
                half = 256 * (ch % 2)
                for j in (0, 1):
                    nc.vector.tensor_tensor(
                        out=y_sb[64 * j : 64 * j + 64, hp, half : half + 256],
                        in0=yb[0:64, 256 * j : 256 * j + 256],
                        in1=rb[64 * j : 64 * j + 64, 256 * j : 256 * j + 256],
                        op=mybir.AluOpType.mult,
                    )
            return run

        def ysb_item(pch):
            def run():
                state[("y", pch)] = ysb_pool.tile(
                    [128, 4, 512], BF16, tag="ysb", name=f"ysb{pch}")
            return run

        def pj_item(pch, qlo=0, qn=512):
            # projection for query cols [512*pch+qlo, 512*pch+qlo+qn)
            def run():
                y_sb = state[("y", pch)]
                for co in range(8):
                    pp = psY.tile([128, 512], F32, tag="yb",
                                  name=f"pp{pch}{co}{qlo}")
                    for ci in range(4):
                        nc.tensor.matmul(
                            pp[:, 0:qn],
                            wp_sb[:, ci, 128 * co : 128 * co + 128],
                            y_sb[:, ci, qlo : qlo + qn],
                            start=(ci == 0),
                            stop=(ci == 3),
                        )
                    o_sb = o_pool.tile([128, 512], BF16, tag="o",
                                       name=f"o{pch}{co}{qlo}")
                    if co % 2 == 0:
                        nc.scalar.activation(o_sb[:, 0:qn], pp[:, 0:qn], AF.Copy)
                    else:
                        nc.vector.tensor_copy(o_sb[:, 0:qn], pp[:, 0:qn])
                    nc.sync.dma_start(
                        outT[128 * co : 128 * co + 128,
                             512 * pch + qlo : 512 * pch + qlo + qn],
                        o_sb[:, 0:qn],
                    )
            return run

        # ---- full-lag schedule: period t emits phase-A columns t
        # interleaved with the attention chunks of period t-1 (whose
        # dependencies are long satisfied), so B's scalar/vector chains
        # hide under A's PE-heavy stretch. Within the B stream, AV(hp)
        # trails scores(hp+1) by one slot (lag-1). ----
        def a_items_for(t):
            items = []
            for co in range(4):
                items.append(a_group(t, "q", co))
            for co in range(4):
                items.append(a_group(t, "k", co))
            for eo in range(4 * t, 4 * t + 4):
                items.append(a_group(t, "v", eo))
            return items

        def b_items_for(c0, c1):
            # lag-1 pipeline over the 8 (ch, hp) slots of chunks c0, c1
            slots = [(c0, hp) for hp in range(4)] + [(c1, hp) for hp in range(4)]
            items = [ysb_item(c0 // 2)]
            prev = None
            for s in slots:
                items.append(sc_item(*s))
                if prev is not None:
                    items.append(avn_item(*prev))
                prev = s
            items.append(avn_item(*prev))
            return items

        def interleave(a_items, b_items):
            na, nb = len(a_items), len(b_items)
            ia = ib = 0
            while ia < na or ib < nb:
                if ia < na and (ib >= nb or ia * nb <= ib * na):
                    a_items[ia]()
                    ia += 1
                else:
                    b_items[ib]()
                    ib += 1

        # period 0: phase A t=0 alone
        interleave(a_items_for(0), [])
        # periods 1..3: A(t) x B(2t-2), B(2t-1) [+ proj of the pair before]
        for t in range(1, 4):
            b_items = b_items_for(2 * t - 2, 2 * t - 1)
            if t >= 2:
                b_items.insert(0, pj_item(t - 2))
            interleave(a_items_for(t), b_items)
        # tail: chunks 6,7; pj2 fills PE early in the tail
        tail = b_items_for(6, 7)
        tail.insert(0, pj_item(2))
        interleave([], tail)
        pj_item(3)()


_BUILD_CACHE = {}


def build_bass(enable_asserts=False):
    key = enable_asserts
    if key in _BUILD_CACHE:
        return _BUILD_CACHE[key]
    nc = bacc.Bacc(
        "TRN2",
        target_bir_lowering=False,
        debug=False,
        enable_asserts=enable_asserts,
    )
    xT = nc.dram_tensor("xT", [C, T], BF16, kind="ExternalInput").ap()
    wqkvT = nc.dram_tensor("wqkvT", [C, 1536], BF16, kind="ExternalInput").ap()
    wprojT = nc.dram_tensor("wprojT", [512, C], BF16, kind="ExternalInput").ap()
    masks = nc.dram_tensor("masks", [2, 128, 128], BF16, kind="ExternalInput").ap()
    outT = nc.dram_tensor("outT", [C, T], BF16, kind="ExternalOutput").ap()

    with tile.TileContext(nc) as tc:
        _build_body(tc, xT, wqkvT, wprojT, masks, outT)
    nc.compile()
    _BUILD_CACHE[key] = nc
    return nc


def make_masks() -> np.ndarray:
    """[2, 128, 128]: mU[kr,qq] = kr >= qq+1 (s0/s1-edge), mL = kr <= qq
    (s3/s2-edge)."""
    kr = np.arange(128)[:, None]
    qq = np.arange(128)[None, :]
    mU = (kr >= qq + 1).astype(np.float32)
    mL = (kr <= qq).astype(np.float32)
    return np.stack([mU, mL]).astype(ml_dtypes.bfloat16)


def kernel(x, W_qkv, W_proj):
    bf16 = ml_dtypes.bfloat16
    x = np.asarray(x, dtype=np.float32)
    W_qkv = np.asarray(W_qkv, dtype=np.float32)
    W_proj = np.asarray(W_proj, dtype=np.float32)

    nc = build_bass()
    wqkvT = W_qkv.T  # [C, 3C]
    wprojT = W_proj.T  # [C, C] (in, out)
    masks_np = make_masks()

    xTb = [np.ascontiguousarray(x[b].T).astype(bf16) for b in range(B)]
    in_maps = []
    for core in range(8):
        b, hh = core // 2, core % 2
        sl = slice(512 * hh, 512 * hh + 512)
        wq = np.concatenate(
            [wqkvT[:, sl], wqkvT[:, 1024 + 512 * hh : 1536 + 512 * hh],
             wqkvT[:, 2048 + 512 * hh : 2560 + 512 * hh]], axis=1
        ).astype(bf16)
        in_maps.append({
            "xT": xTb[b],
            "wqkvT": np.ascontiguousarray(wq),
            "wprojT": np.ascontiguousarray(wprojT[sl, :]).astype(bf16),
            "masks": masks_np,
        })
    res = bass_utils.run_bass_kernel_spmd(nc, in_maps, core_ids=list(range(8)))
    kernel.last_run_results = res

    out = np.empty((B, T, C), dtype=np.float32)
    for b in range(B):
        out[b] = (res.results[2 * b]["outT"].astype(np.float32)
                  + res.results[2 * b + 1]["outT"].astype(np.float32)).T
    return out


kernel.last_run_results = None
